# revision 18
# baseline (speedup 1.0000x reference)
"""Trainium2 Bass kernel for nn_GCBlock (gnn_message_passing) — v5.

Same math as v4 (fp16 matmul pipeline, host LN-affine + residual), plus:
- 3-stage software-pipelined emission: front(g) / stageA(g-1) / fc+tail(g-2)
  per emission step, so every engine queue always holds ready work and the
  PE stream stays dense (HAM warm).
- h shipped as fp16; raw LN sums/sumsq shipped per group; the host computes
  mean/var/rstd/negmr in fp64-ish fp32 and applies
  y = x + (h*rstd + negmr)*alpha + beta.
- DMA transposes batched 2 groups per instruction (halves SP queue load).
"""
import numpy as np

B, V, T, J = 2048, 66, 256, 22
N_CORES = 8
BL = B // N_CORES          # 256 samples per core
NB = 8                     # samples per group
NG = BL // NB              # 32 groups
FD = NB * V                # 528 batched free dim
FD2 = 2 * FD               # both time halves side by side
HC = FD // 2               # 264 per col-half
PW = 1024                  # psum tile width (2 banks); c-blocks at 0 / 512
XG = 4                     # groups per natural-x batch DMA
W2SCALE = 64.0             # host-folds: W2 *= 64, g1 pattern /= 64

_NC_CACHE = {}


def _build_nc():
    if "nc" in _NC_CACHE:
        return _NC_CACHE["nc"]
    import concourse.bacc as bacc
    import concourse.mybir as mybir
    import concourse.tile as tile

    f32 = mybir.dt.float32
    f16 = mybir.dt.float16
    bf16 = mybir.dt.bfloat16
    Alu = mybir.AluOpType
    Act = mybir.ActivationFunctionType

    nc = bacc.Bacc("TRN2", target_bir_lowering=False, debug=False,
                   num_devices=N_CORES)

    xs = nc.dram_tensor("xs", [BL, V, T], f16, kind="ExternalInput").ap()
    xs2 = nc.dram_tensor("xs2", [V, BL * T], f16, kind="ExternalInput").ap()
    alt = nc.dram_tensor("alt", [V, BL * V], f16, kind="ExternalInput").ap()
    gp = nc.dram_tensor("gp", [128, NG * 16], f16, kind="ExternalInput").ap()
    atc = nc.dram_tensor("atc", [2, 128, FD2], f16, kind="ExternalInput").ap()
    wq = nc.dram_tensor("wq", [2, 2, 2, 128, 128], f16, kind="ExternalInput").ap()
    smat = nc.dram_tensor("smat", [4, 128, 128], f16, kind="ExternalInput").ap()
    ys = nc.dram_tensor("ys", [2, 128, NG * FD], f16, kind="ExternalOutput").ap()

    with tile.TileContext(nc) as tc:
        import contextlib
        with contextlib.ExitStack() as ctx:
            cpool = ctx.enter_context(tc.tile_pool(name="consts", bufs=1))
            xbpool = ctx.enter_context(tc.tile_pool(name="xbatch", bufs=2))
            wpool = ctx.enter_context(tc.tile_pool(name="work", bufs=4))
            stpool = ctx.enter_context(tc.tile_pool(name="stats", bufs=4))
            pp = ctx.enter_context(tc.tile_pool(name="ps", bufs=1, space="PSUM"))

            # ---- constants (loaded once) ----
            c_at = [cpool.tile([128, FD2], f16, name=f"cat{d}", tag=f"cat{d}")
                    for d in range(2)]
            for d in range(2):
                nc.sync.dma_start(c_at[d][:], atc[d])
            c_wq = [[[cpool.tile([128, 128], f16, name=f"cwq{w}{kh}{F}",
                                 tag=f"cwq{w}{kh}{F}")
                      for F in range(2)] for kh in range(2)] for w in range(2)]
            for w in range(2):
                for kh in range(2):
                    for F in range(2):
                        nc.sync.dma_start(c_wq[w][kh][F][:], wq[w, kh, F])
            c_sm = [cpool.tile([128, 128], f16, name=f"csm{k}", tag=f"csm{k}")
                    for k in range(4)]
            for k in range(4):
                nc.sync.dma_start(c_sm[k][:], smat[k])
            c_gp = cpool.tile([128, NG * 16], f16, name="cgp", tag="cgp")
            nc.sync.dma_start(c_gp[:], gp[:])
            c_alt = cpool.tile([V, BL * V], f16, name="calt", tag="calt")
            nc.sync.dma_start(c_alt[:], alt[:])

            EYE_D, EYE_U, ZS_D, ZS_U = 0, 1, 2, 3

            def pslice(t, c, lo, hi):
                return t[:, 512 * c + lo:512 * c + hi]

            # cross-stage state
            st = {}

            def issue_sxt(p):
                # prefetch transposed x for group pair p (groups 2p, 2p+1);
                # tile layout [128, (q, h, f)] so each group's view is packed
                if p >= NG // 2:
                    return
                st[("sxt", p)] = sxt = wpool.tile(
                    [128, 2 * FD2], f16, name="t02", tag="sxt", bufs=3)
                for q in range(2):
                    for h in range(2):
                        eng = nc.sync if h == 0 else nc.scalar
                        eng.dma_start(
                            sxt[:, FD2 * q + FD * h:FD2 * q + FD * (h + 1)],
                            xs[(2 * p + q) * NB:(2 * p + q + 1) * NB, :,
                               128 * h:128 * (h + 1)]
                            .rearrange("b v t -> (b v) t"),
                            transpose=True)

            def front(g):
                if g % XG == 0:
                    st[("xng", g // XG)] = xng = xbpool.tile(
                        [V, XG * NB * T], f16, name="t01", tag="xng")
                    nc.scalar.dma_start(
                        xng[:], xs2[:, g * NB * T:(g + XG) * NB * T])
                if g % 2 == 0:
                    issue_sxt(g // 2 + 1)
                # packed per-group view [128, (h, f)]
                sxt = st[("sxt", g // 2)]
                q = g % 2
                sxtg = sxt[:, FD2 * q:FD2 * (q + 1)]

                g1r = wpool.tile([128, FD], f16, name="t03", tag="g1r")
                g3r = wpool.tile([128, FD], f16, name="t04", tag="g3r")
                gsl = c_gp[:, g * 16:(g + 1) * 16]
                nc.scalar.copy(
                    g1r[:].rearrange("p (n v) -> p n v", n=NB),
                    gsl[:, 0:NB].unsqueeze(2).broadcast_to([128, NB, V]))
                nc.scalar.copy(
                    g3r[:].rearrange("p (n v) -> p n v", n=NB),
                    gsl[:, NB:16].unsqueeze(2).broadcast_to([128, NB, V]))

                g1s = wpool.tile([128, FD2], f16, name="t05", tag="g1s")
                g3s = wpool.tile([128, FD2], f16, name="t06", tag="g3s")
                ulo = wpool.tile([128, FD2], f16, name="t07", tag="ulo")
                uhi = wpool.tile([128, FD2], f16, name="t08", tag="uhi")
                sxt3 = sxtg.rearrange("p (h f) -> p h f", h=2)
                g1b = g1r[:].unsqueeze(1).broadcast_to([128, 2, FD])
                g3b = g3r[:].unsqueeze(1).broadcast_to([128, 2, FD])
                nc.vector.tensor_tensor(
                    g1s[:].rearrange("p (h f) -> p h f", h=2), g1b, sxt3,
                    Alu.mult)
                nc.vector.tensor_tensor(
                    g3s[:].rearrange("p (h f) -> p h f", h=2), g3b, sxt3,
                    Alu.mult)
                nc.vector.tensor_tensor(ulo[:], g3s[:], c_at[0][:], Alu.mult)
                nc.vector.tensor_tensor(uhi[:], g3s[:], c_at[1][:], Alu.mult)
                st[("g1s", g)] = g1s
                st[("ulo", g)] = ulo
                st[("uhi", g)] = uhi

            def stage_a(g):
                ulo, uhi = st.pop(("ulo", g)), st.pop(("uhi", g))
                xng = st[("xng", g // XG)]
                pXM = [pp.tile([128, PW], f32, name="t09", tag="pp", bufs=4)
                       for _ in range(2)]
                xoff = (g % XG) * NB * T
                for h in range(2):
                    for c in range(2):
                        for i in range(4):
                            s = 4 * c + i
                            b = g * NB + s
                            lhs = xng[:, xoff + s * T + 128 * h:
                                      xoff + s * T + 128 * (h + 1)]
                            nc.tensor.matmul(
                                pslice(pXM[h], c, 66 * i, 66 * (i + 1)),
                                lhs, c_alt[:, b * V:(b + 1) * V],
                                start=(i == 0), stop=False,
                                skip_group_check=True)
                        sl_ = slice(FD * h + HC * c, FD * h + HC * (c + 1))
                        out_c = pslice(pXM[h], c, 0, HC)
                        nc.tensor.matmul(out_c, c_sm[EYE_D][:],
                                         ulo[:, sl_], start=False,
                                         stop=False, skip_group_check=True)
                        nc.tensor.matmul(out_c, c_sm[EYE_U][:],
                                         uhi[:, sl_], start=False,
                                         stop=False, skip_group_check=True)
                        if h == 1:
                            osl = slice(HC * c, HC * (c + 1))
                            nc.tensor.matmul(out_c, c_sm[ZS_D][:],
                                             ulo[:, osl], start=False,
                                             stop=True, skip_group_check=True)
                        else:
                            osl = slice(FD + HC * c, FD + HC * (c + 1))
                            nc.tensor.matmul(out_c, c_sm[ZS_U][:],
                                             uhi[:, osl], start=False,
                                             stop=True,
                                             skip_group_check=True)
                xms = wpool.tile([128, FD2], f16, name="t10", tag="xms")
                for h in range(2):
                    nc.scalar.copy(
                        xms[:, FD * h:FD * (h + 1)]
                        .rearrange("p (c f) -> p c f", c=2),
                        pXM[h][:].rearrange("p (c f) -> p c f", c=2)
                        [:, :, 0:HC])
                st[("xms", g)] = xms

            def fc_tail(g):
                xms = st.pop(("xms", g))
                g1s = st.pop(("g1s", g))
                pH = [pp.tile([128, PW], f32, name="t11", tag="pp", bufs=4)
                      for _ in range(2)]
                for F in range(2):
                    for c in range(2):
                        first = True
                        for kh in range(2):
                            sl_ = slice(FD * kh + HC * c,
                                        FD * kh + HC * (c + 1))
                            for w, stream in ((0, xms), (1, g1s)):
                                nc.tensor.matmul(
                                    pslice(pH[F], c, 0, HC),
                                    c_wq[w][kh][F][:],
                                    stream[:, sl_],
                                    start=first,
                                    stop=(kh == 1 and w == 1))
                                first = False

                sH = [wpool.tile([128, FD], f16, name="t12", tag=f"sh{F}")
                      for F in range(2)]
                for F in range(2):
                    pHs = pH[F][:].rearrange("p (c f) -> p c f", c=2)[:, :, 0:HC]
                    dst = sH[F][:].rearrange("p (c f) -> p c f", c=2)
                    if F == 0:
                        nc.scalar.copy(dst, pHs)
                    else:
                        nc.vector.tensor_copy(dst, pHs)
                    nc.gpsimd.dma_start(ys[F, :, g * FD:(g + 1) * FD],
                                        sH[F][:])

            issue_sxt(0)
            for gg in range(NG + 2):
                if gg < NG:
                    front(gg)
                if 1 <= gg <= NG:
                    stage_a(gg - 1)
                if gg >= 2:
                    fc_tail(gg - 2)

    nc.compile()
    _NC_CACHE["nc"] = nc
    return nc


def _gate_np(x, mlp, if_make_dynamic, tau):
    """Replicate the reference gating exactly (jax fp32 on CPU)."""
    import jax
    import jax.numpy as jnp

    xj = jnp.asarray(x)
    prob = xj.mean(axis=1) @ jnp.asarray(mlp)
    if if_make_dynamic:
        u = jax.random.uniform(jax.random.key(42), prob.shape,
                               minval=1e-10, maxval=1.0)
        gumbel = -jnp.log(-jnp.log(u))
        soft = jax.nn.softmax((prob + gumbel) / tau, axis=-1)
        # forward value of (hard + soft - stop_grad(soft)) is exactly hard;
        # return the exact one-hot so 0/1 gates survive integer indexing
        gate = jax.nn.one_hot(jnp.argmax(soft, axis=-1), prob.shape[-1],
                              dtype=soft.dtype)
    else:
        gate = jnp.zeros_like(prob).at[:, 0].set(1.0)
    return np.asarray(gate, dtype=np.float32)


def kernel(x, mlp, adj_j, adj_t, adj_jc, adj_tj, fc_w, fc_b, alpha, beta,
           if_make_dynamic, tau):
    from concourse.bass_utils import run_bass_kernel_spmd

    x = np.asarray(x, dtype=np.float32)
    mlp = np.asarray(mlp, dtype=np.float32)
    adj_j = np.asarray(adj_j, dtype=np.float32)
    adj_t = np.asarray(adj_t, dtype=np.float32)
    adj_jc = np.asarray(adj_jc, dtype=np.float32)
    adj_tj = np.asarray(adj_tj, dtype=np.float32)
    fc_w = np.asarray(fc_w, dtype=np.float32)
    alpha_v = np.asarray(alpha, dtype=np.float32).reshape(V)
    beta_v = np.asarray(beta, dtype=np.float32).reshape(V)

    gate = _gate_np(x, mlp, if_make_dynamic, tau)
    g1, g2, g3 = gate[:, 1], gate[:, 2], gate[:, 3]

    A1 = np.kron(adj_j, np.eye(3, dtype=np.float32))
    A3 = np.zeros((V, V), dtype=np.float32)
    for j in range(J):
        A3[3 * j:3 * j + 3, 3 * j:3 * j + 3] = adj_jc[j]
    alt2 = np.stack([A1.T.copy(), (A1 + A3).T.copy()]).astype(np.float16)
    alt_all = alt2[g2.astype(np.int64)]                   # [B, V, V]

    idx = np.arange(T)
    band = (np.abs(idx[:, None] - idx[None, :]) == 1).astype(np.float32)
    M2 = adj_t * band
    W2 = (fc_w @ M2) * W2SCALE

    wq = np.zeros((2, 2, 2, 128, 128), dtype=np.float32)
    for kh in range(2):
        for F in range(2):
            wq[0, kh, F] = fc_w[128 * F:128 * (F + 1),
                                128 * kh:128 * (kh + 1)].T
            wq[1, kh, F] = W2[128 * F:128 * (F + 1),
                              128 * kh:128 * (kh + 1)].T

    alo_p = np.zeros((T, V), dtype=np.float32)
    ahi_p = np.zeros((T, V), dtype=np.float32)
    alo_p[:T - 1, :] = adj_tj[:, np.arange(1, T), np.arange(0, T - 1)].T
    ahi_p[1:, :] = adj_tj[:, np.arange(0, T - 1), np.arange(1, T)].T
    atc = np.zeros((2, 128, FD2), dtype=np.float32)
    for h in range(2):
        atc[0, :, FD * h:FD * (h + 1)] = np.tile(
            alo_p[128 * h:128 * (h + 1)], (1, NB))
        atc[1, :, FD * h:FD * (h + 1)] = np.tile(
            ahi_p[128 * h:128 * (h + 1)], (1, NB))

    smat = np.zeros((4, 128, 128), dtype=np.float32)
    smat[0] = np.eye(128, k=1)      # EYE_D: out[p] = u[p-1]
    smat[1] = np.eye(128, k=-1)     # EYE_U: out[p] = u[p+1]
    smat[2][127, 0] = 1.0           # ZS_D seam: h1 p0 <- u_lo[h0][127]
    smat[3][0, 127] = 1.0           # ZS_U seam: h0 p127 <- u_hi[h1][0]

    x_f16 = x.astype(np.float16)
    atc_f16 = atc.astype(np.float16)
    wq_f16 = wq.astype(np.float16)
    smat_f16 = smat.astype(np.float16)

    in_maps = []
    for cidx in range(N_CORES):
        sl_ = slice(cidx * BL, (cidx + 1) * BL)
        gp_c = np.zeros((NG, 16), dtype=np.float32)
        gp_c[:, 0:NB] = g1[sl_].reshape(NG, NB) / W2SCALE
        gp_c[:, NB:16] = g3[sl_].reshape(NG, NB)
        gp_full = np.broadcast_to(gp_c.reshape(1, NG * 16),
                                  (128, NG * 16)).astype(np.float16)
        in_maps.append(dict(
            xs=np.ascontiguousarray(x_f16[sl_]),
            xs2=np.ascontiguousarray(
                x_f16[sl_].transpose(1, 0, 2)).reshape(V, BL * T),
            alt=np.ascontiguousarray(
                alt_all[sl_].transpose(1, 0, 2)).reshape(V, BL * V),
            gp=gp_full, atc=atc_f16, wq=wq_f16, smat=smat_f16,
        ))

    nc = _build_nc()
    res = run_bass_kernel_spmd(nc, in_maps, core_ids=list(range(N_CORES)),
                               **_RUN_KW)
    _LAST_RES.clear()
    _LAST_RES["res"] = res

    # host tail: LN stats from shipped fp16 h (centered, fp32, clamped),
    # then y = x + (h - m)/sqrt(var+eps) * alpha + beta
    out = np.empty((B, V, T), dtype=np.float32)
    for cidx in range(N_CORES):
        yt = np.asarray(res.results[cidx]["ys"]).astype(np.float32)
        h = yt.reshape(2, 128, NG, NB, V).transpose(2, 3, 4, 0, 1) \
            .reshape(BL, V, T)
        m = h.mean(axis=1, keepdims=True)
        var = h.var(axis=1, keepdims=True)
        nv = (h - m) / np.sqrt(var + 1e-5)
        nv = nv * alpha_v[None, :, None] + beta_v[None, :, None]
        out[cidx * BL:(cidx + 1) * BL] = x[cidx * BL:(cidx + 1) * BL] + nv
    return out


_RUN_KW = {}
_LAST_RES = {}


# revision 21
# speedup vs baseline: 1.3374x; 1.3374x over previous
"""Trainium2 Bass kernel for nn_GCBlock (gnn_message_passing) — v5.

Same math as v4 (fp16 matmul pipeline, host LN-affine + residual), plus:
- 3-stage software-pipelined emission: front(g) / stageA(g-1) / fc+tail(g-2)
  per emission step, so every engine queue always holds ready work and the
  PE stream stays dense (HAM warm).
- h shipped as fp16; raw LN sums/sumsq shipped per group; the host computes
  mean/var/rstd/negmr in fp64-ish fp32 and applies
  y = x + (h*rstd + negmr)*alpha + beta.
- DMA transposes batched 2 groups per instruction (halves SP queue load).
"""
import numpy as np

B, V, T, J = 2048, 66, 256, 22
N_CORES = 8
BL = B // N_CORES          # 256 samples per core
NB = 8                     # samples per group
NG = BL // NB              # 32 groups
FD = NB * V                # 528 batched free dim
FD2 = 2 * FD               # both time halves side by side
HC = FD // 2               # 264 per col-half
PW = 1024                  # psum tile width (2 banks); c-blocks at 0 / 512
XG = 4                     # groups per natural-x batch DMA
W2SCALE = 64.0             # host-folds: W2 *= 64, g1 pattern /= 64

_NC_CACHE = {}


def _build_nc():
    if "nc" in _NC_CACHE:
        return _NC_CACHE["nc"]
    import concourse.bacc as bacc
    import concourse.mybir as mybir
    import concourse.tile as tile

    f32 = mybir.dt.float32
    f16 = mybir.dt.float16
    bf16 = mybir.dt.bfloat16
    Alu = mybir.AluOpType
    Act = mybir.ActivationFunctionType

    nc = bacc.Bacc("TRN2", target_bir_lowering=False, debug=False,
                   num_devices=N_CORES)

    xs = nc.dram_tensor("xs", [BL, V, T], f16, kind="ExternalInput").ap()
    xs2 = nc.dram_tensor("xs2", [V, BL * T], f16, kind="ExternalInput").ap()
    alt = nc.dram_tensor("alt", [V, BL * V], f16, kind="ExternalInput").ap()
    gp = nc.dram_tensor("gp", [128, NG * 16], f16, kind="ExternalInput").ap()
    atc = nc.dram_tensor("atc", [2, 128, FD2], f16, kind="ExternalInput").ap()
    wq = nc.dram_tensor("wq", [2, 2, 2, 128, 128], f16, kind="ExternalInput").ap()
    smat = nc.dram_tensor("smat", [4, 128, 128], f16, kind="ExternalInput").ap()
    ys = nc.dram_tensor("ys", [2, 128, NG * FD], f16, kind="ExternalOutput").ap()

    with tile.TileContext(nc) as tc:
        import contextlib
        with contextlib.ExitStack() as ctx:
            cpool = ctx.enter_context(tc.tile_pool(name="consts", bufs=1))
            xbpool = ctx.enter_context(tc.tile_pool(name="xbatch", bufs=2))
            wpool = ctx.enter_context(tc.tile_pool(name="work", bufs=4))
            stpool = ctx.enter_context(tc.tile_pool(name="stats", bufs=4))
            pp = ctx.enter_context(tc.tile_pool(name="ps", bufs=1, space="PSUM"))

            # ---- constants (loaded once) ----
            c_at = [cpool.tile([128, FD2], f16, name=f"cat{d}", tag=f"cat{d}")
                    for d in range(2)]
            for d in range(2):
                nc.sync.dma_start(c_at[d][:], atc[d])
            c_wq = [[[cpool.tile([128, 128], f16, name=f"cwq{w}{kh}{F}",
                                 tag=f"cwq{w}{kh}{F}")
                      for F in range(2)] for kh in range(2)] for w in range(2)]
            for w in range(2):
                for kh in range(2):
                    for F in range(2):
                        nc.sync.dma_start(c_wq[w][kh][F][:], wq[w, kh, F])
            c_sm = [cpool.tile([128, 128], f16, name=f"csm{k}", tag=f"csm{k}")
                    for k in range(4)]
            for k in range(4):
                nc.sync.dma_start(c_sm[k][:], smat[k])
            c_gp = cpool.tile([128, NG * 16], f16, name="cgp", tag="cgp")
            nc.sync.dma_start(c_gp[:], gp[:])
            c_alt = cpool.tile([V, BL * V], f16, name="calt", tag="calt")
            nc.sync.dma_start(c_alt[:], alt[:])

            EYE_D, EYE_U, ZS_D, ZS_U = 0, 1, 2, 3

            def pslice(t, c, lo, hi):
                return t[:, 512 * c + lo:512 * c + hi]

            # cross-stage state
            st = {}

            def issue_sxt(p):
                # prefetch transposed x for group pair p (groups 2p, 2p+1);
                # tile layout [128, (q, h, f)] so each group's view is packed
                if p >= NG // 2:
                    return
                # layout [128, (h, q, f)]: one 2-group xbar transpose per h
                # (single queue — concurrent xbar transposes are unsafe)
                st[("sxt", p)] = sxt = wpool.tile(
                    [128, 2 * FD2], f16, name="t02", tag="sxt", bufs=3)
                for h in range(2):
                    nc.sync.dma_start(
                        sxt[:, FD2 * h:FD2 * (h + 1)],
                        xs[2 * p * NB:(2 * p + 2) * NB, :,
                           128 * h:128 * (h + 1)]
                        .rearrange("b v t -> (b v) t"),
                        transpose=True)

            def front(g):
                if g % XG == 0:
                    st[("xng", g // XG)] = xng = xbpool.tile(
                        [V, XG * NB * T], f16, name="t01", tag="xng")
                    nc.scalar.dma_start(
                        xng[:], xs2[:, g * NB * T:(g + XG) * NB * T])
                if g % 2 == 0:
                    issue_sxt(g // 2 + 1)
                # per-group view [128, 2(h), FD] (strided over the q pairing)
                sxt = st[("sxt", g // 2)]
                q = g % 2

                g1r = wpool.tile([128, FD], f16, name="t03", tag="g1r")
                g3r = wpool.tile([128, FD], f16, name="t04", tag="g3r")
                gsl = c_gp[:, g * 16:(g + 1) * 16]
                nc.scalar.copy(
                    g1r[:].rearrange("p (n v) -> p n v", n=NB),
                    gsl[:, 0:NB].unsqueeze(2).broadcast_to([128, NB, V]))
                nc.scalar.copy(
                    g3r[:].rearrange("p (n v) -> p n v", n=NB),
                    gsl[:, NB:16].unsqueeze(2).broadcast_to([128, NB, V]))

                g1s = wpool.tile([128, FD2], f16, name="t05", tag="g1s")
                g3s = wpool.tile([128, FD2], f16, name="t06", tag="g3s")
                ulo = wpool.tile([128, FD2], f16, name="t07", tag="ulo")
                uhi = wpool.tile([128, FD2], f16, name="t08", tag="uhi")
                sxt3 = sxt[:].rearrange("p (h q f) -> p h q f",
                                        h=2, q=2)[:, :, q, :]
                g1b = g1r[:].unsqueeze(1).broadcast_to([128, 2, FD])
                g3b = g3r[:].unsqueeze(1).broadcast_to([128, 2, FD])
                nc.vector.tensor_tensor(
                    g1s[:].rearrange("p (h f) -> p h f", h=2), g1b, sxt3,
                    Alu.mult)
                nc.vector.tensor_tensor(
                    g3s[:].rearrange("p (h f) -> p h f", h=2), g3b, sxt3,
                    Alu.mult)
                nc.vector.tensor_tensor(ulo[:], g3s[:], c_at[0][:], Alu.mult)
                nc.vector.tensor_tensor(uhi[:], g3s[:], c_at[1][:], Alu.mult)
                st[("g1s", g)] = g1s
                st[("ulo", g)] = ulo
                st[("uhi", g)] = uhi

            def stage_a(g):
                ulo, uhi = st.pop(("ulo", g)), st.pop(("uhi", g))
                xng = st[("xng", g // XG)]
                pXM = [pp.tile([128, PW], f32, name="t09", tag="pp", bufs=4)
                       for _ in range(2)]
                xoff = (g % XG) * NB * T
                for h in range(2):
                    for c in range(2):
                        for i in range(4):
                            s = 4 * c + i
                            b = g * NB + s
                            lhs = xng[:, xoff + s * T + 128 * h:
                                      xoff + s * T + 128 * (h + 1)]
                            nc.tensor.matmul(
                                pslice(pXM[h], c, 66 * i, 66 * (i + 1)),
                                lhs, c_alt[:, b * V:(b + 1) * V],
                                start=(i == 0), stop=False,
                                skip_group_check=True)
                        sl_ = slice(FD * h + HC * c, FD * h + HC * (c + 1))
                        out_c = pslice(pXM[h], c, 0, HC)
                        nc.tensor.matmul(out_c, c_sm[EYE_D][:],
                                         ulo[:, sl_], start=False,
                                         stop=False, skip_group_check=True)
                        nc.tensor.matmul(out_c, c_sm[EYE_U][:],
                                         uhi[:, sl_], start=False,
                                         stop=False, skip_group_check=True)
                        if h == 1:
                            osl = slice(HC * c, HC * (c + 1))
                            nc.tensor.matmul(out_c, c_sm[ZS_D][:],
                                             ulo[:, osl], start=False,
                                             stop=True, skip_group_check=True)
                        else:
                            osl = slice(FD + HC * c, FD + HC * (c + 1))
                            nc.tensor.matmul(out_c, c_sm[ZS_U][:],
                                             uhi[:, osl], start=False,
                                             stop=True,
                                             skip_group_check=True)
                xms = wpool.tile([128, FD2], f16, name="t10", tag="xms")
                for h in range(2):
                    nc.scalar.copy(
                        xms[:, FD * h:FD * (h + 1)]
                        .rearrange("p (c f) -> p c f", c=2),
                        pXM[h][:].rearrange("p (c f) -> p c f", c=2)
                        [:, :, 0:HC])
                st[("xms", g)] = xms

            def fc_tail(g):
                xms = st.pop(("xms", g))
                g1s = st.pop(("g1s", g))
                pH = [pp.tile([128, PW], f32, name="t11", tag="pp", bufs=4)
                      for _ in range(2)]
                for F in range(2):
                    for c in range(2):
                        first = True
                        for kh in range(2):
                            sl_ = slice(FD * kh + HC * c,
                                        FD * kh + HC * (c + 1))
                            for w, stream in ((0, xms), (1, g1s)):
                                nc.tensor.matmul(
                                    pslice(pH[F], c, 0, HC),
                                    c_wq[w][kh][F][:],
                                    stream[:, sl_],
                                    start=first,
                                    stop=(kh == 1 and w == 1))
                                first = False

                sH = [wpool.tile([128, FD], f16, name="t12", tag=f"sh{F}")
                      for F in range(2)]
                for F in range(2):
                    pHs = pH[F][:].rearrange("p (c f) -> p c f", c=2)[:, :, 0:HC]
                    dst = sH[F][:].rearrange("p (c f) -> p c f", c=2)
                    if F == 0:
                        nc.scalar.copy(dst, pHs)
                    else:
                        nc.vector.tensor_copy(dst, pHs)
                    nc.gpsimd.dma_start(ys[F, :, g * FD:(g + 1) * FD],
                                        sH[F][:])

            issue_sxt(0)
            for gg in range(NG + 2):
                if gg < NG:
                    front(gg)
                if 1 <= gg <= NG:
                    stage_a(gg - 1)
                if gg >= 2:
                    fc_tail(gg - 2)

    nc.compile()
    _NC_CACHE["nc"] = nc
    return nc


def _gate_np(x, mlp, if_make_dynamic, tau):
    """Replicate the reference gating exactly (jax fp32 on CPU)."""
    import jax
    import jax.numpy as jnp

    xj = jnp.asarray(x)
    prob = xj.mean(axis=1) @ jnp.asarray(mlp)
    if if_make_dynamic:
        u = jax.random.uniform(jax.random.key(42), prob.shape,
                               minval=1e-10, maxval=1.0)
        gumbel = -jnp.log(-jnp.log(u))
        soft = jax.nn.softmax((prob + gumbel) / tau, axis=-1)
        # forward value of (hard + soft - stop_grad(soft)) is exactly hard;
        # return the exact one-hot so 0/1 gates survive integer indexing
        gate = jax.nn.one_hot(jnp.argmax(soft, axis=-1), prob.shape[-1],
                              dtype=soft.dtype)
    else:
        gate = jnp.zeros_like(prob).at[:, 0].set(1.0)
    return np.asarray(gate, dtype=np.float32)


def kernel(x, mlp, adj_j, adj_t, adj_jc, adj_tj, fc_w, fc_b, alpha, beta,
           if_make_dynamic, tau):
    from concourse.bass_utils import run_bass_kernel_spmd

    x = np.asarray(x, dtype=np.float32)
    mlp = np.asarray(mlp, dtype=np.float32)
    adj_j = np.asarray(adj_j, dtype=np.float32)
    adj_t = np.asarray(adj_t, dtype=np.float32)
    adj_jc = np.asarray(adj_jc, dtype=np.float32)
    adj_tj = np.asarray(adj_tj, dtype=np.float32)
    fc_w = np.asarray(fc_w, dtype=np.float32)
    alpha_v = np.asarray(alpha, dtype=np.float32).reshape(V)
    beta_v = np.asarray(beta, dtype=np.float32).reshape(V)

    gate = _gate_np(x, mlp, if_make_dynamic, tau)
    g1, g2, g3 = gate[:, 1], gate[:, 2], gate[:, 3]

    A1 = np.kron(adj_j, np.eye(3, dtype=np.float32))
    A3 = np.zeros((V, V), dtype=np.float32)
    for j in range(J):
        A3[3 * j:3 * j + 3, 3 * j:3 * j + 3] = adj_jc[j]
    alt2 = np.stack([A1.T.copy(), (A1 + A3).T.copy()]).astype(np.float16)
    alt_all = alt2[g2.astype(np.int64)]                   # [B, V, V]

    idx = np.arange(T)
    band = (np.abs(idx[:, None] - idx[None, :]) == 1).astype(np.float32)
    M2 = adj_t * band
    W2 = (fc_w @ M2) * W2SCALE

    wq = np.zeros((2, 2, 2, 128, 128), dtype=np.float32)
    for kh in range(2):
        for F in range(2):
            wq[0, kh, F] = fc_w[128 * F:128 * (F + 1),
                                128 * kh:128 * (kh + 1)].T
            wq[1, kh, F] = W2[128 * F:128 * (F + 1),
                              128 * kh:128 * (kh + 1)].T

    alo_p = np.zeros((T, V), dtype=np.float32)
    ahi_p = np.zeros((T, V), dtype=np.float32)
    alo_p[:T - 1, :] = adj_tj[:, np.arange(1, T), np.arange(0, T - 1)].T
    ahi_p[1:, :] = adj_tj[:, np.arange(0, T - 1), np.arange(1, T)].T
    atc = np.zeros((2, 128, FD2), dtype=np.float32)
    for h in range(2):
        atc[0, :, FD * h:FD * (h + 1)] = np.tile(
            alo_p[128 * h:128 * (h + 1)], (1, NB))
        atc[1, :, FD * h:FD * (h + 1)] = np.tile(
            ahi_p[128 * h:128 * (h + 1)], (1, NB))

    smat = np.zeros((4, 128, 128), dtype=np.float32)
    smat[0] = np.eye(128, k=1)      # EYE_D: out[p] = u[p-1]
    smat[1] = np.eye(128, k=-1)     # EYE_U: out[p] = u[p+1]
    smat[2][127, 0] = 1.0           # ZS_D seam: h1 p0 <- u_lo[h0][127]
    smat[3][0, 127] = 1.0           # ZS_U seam: h0 p127 <- u_hi[h1][0]

    x_f16 = x.astype(np.float16)
    atc_f16 = atc.astype(np.float16)
    wq_f16 = wq.astype(np.float16)
    smat_f16 = smat.astype(np.float16)

    in_maps = []
    for cidx in range(N_CORES):
        sl_ = slice(cidx * BL, (cidx + 1) * BL)
        gp_c = np.zeros((NG, 16), dtype=np.float32)
        gp_c[:, 0:NB] = g1[sl_].reshape(NG, NB) / W2SCALE
        gp_c[:, NB:16] = g3[sl_].reshape(NG, NB)
        gp_full = np.broadcast_to(gp_c.reshape(1, NG * 16),
                                  (128, NG * 16)).astype(np.float16)
        in_maps.append(dict(
            xs=np.ascontiguousarray(x_f16[sl_]),
            xs2=np.ascontiguousarray(
                x_f16[sl_].transpose(1, 0, 2)).reshape(V, BL * T),
            alt=np.ascontiguousarray(
                alt_all[sl_].transpose(1, 0, 2)).reshape(V, BL * V),
            gp=gp_full, atc=atc_f16, wq=wq_f16, smat=smat_f16,
        ))

    nc = _build_nc()
    res = run_bass_kernel_spmd(nc, in_maps, core_ids=list(range(N_CORES)),
                               **_RUN_KW)
    _LAST_RES.clear()
    _LAST_RES["res"] = res

    # host tail: LN stats from shipped fp16 h (centered, fp32, clamped),
    # then y = x + (h - m)/sqrt(var+eps) * alpha + beta
    out = np.empty((B, V, T), dtype=np.float32)
    for cidx in range(N_CORES):
        yt = np.asarray(res.results[cidx]["ys"]).astype(np.float32)
        h = yt.reshape(2, 128, NG, NB, V).transpose(2, 3, 4, 0, 1) \
            .reshape(BL, V, T)
        m = h.mean(axis=1, keepdims=True)
        var = h.var(axis=1, keepdims=True)
        nv = (h - m) / np.sqrt(var + 1e-5)
        nv = nv * alpha_v[None, :, None] + beta_v[None, :, None]
        out[cidx * BL:(cidx + 1) * BL] = x[cidx * BL:(cidx + 1) * BL] + nv
    return out


_RUN_KW = {}
_LAST_RES = {}


# revision 23
# speedup vs baseline: 1.3960x; 1.0438x over previous
"""Trainium2 Bass kernel for nn_GCBlock (gnn_message_passing) — v5.

fp16 matmul pipeline in a transposed (time-on-partition) layout:
- joint-mix (g2 folded into per-sample AL^T, 2 host variants), adj_t band
  folded into W2 = fc_w @ M2 as a second FC stream on g1*xT, adj_tj band
  via shift-commuted coefficient prefold with shift matmuls accumulating
  straight into the joint-mix PSUM.
- 3-stage software-pipelined emission: front(g) / stageA(g-1) / fc+tail(g-2)
  per emission step, so every engine queue always holds ready work and the
  PE stream stays dense (HAM warm).
- xT via xbar DMA-transpose, batched 2 groups per instruction on a single
  HWDGE queue (concurrent xbar transposes are unsafe), prefetched one pair
  ahead.
- device ships h (fp16, transposed); the host computes LN stats (centered,
  fp32) and applies y = x + (h - m)/sqrt(var+eps)*alpha + beta. This keeps
  the LN tail off the device and the residual in full fp32.
- gate is the exact one-hot forward value (hard + soft - stop_grad(soft)
  == hard), so 0/1 gates survive integer indexing on the host.
"""
import numpy as np

B, V, T, J = 2048, 66, 256, 22
N_CORES = 8
BL = B // N_CORES          # 256 samples per core
NB = 8                     # samples per group
NG = BL // NB              # 32 groups
FD = NB * V                # 528 batched free dim
FD2 = 2 * FD               # both time halves side by side
HC = FD // 2               # 264 per col-half
PW = 1024                  # psum tile width (2 banks); c-blocks at 0 / 512
XG = 4                     # groups per natural-x batch DMA
W2SCALE = 64.0             # host-folds: W2 *= 64, g1 pattern /= 64

_NC_CACHE = {}


def _build_nc():
    if "nc" in _NC_CACHE:
        return _NC_CACHE["nc"]
    import concourse.bacc as bacc
    import concourse.mybir as mybir
    import concourse.tile as tile

    f32 = mybir.dt.float32
    f16 = mybir.dt.float16
    bf16 = mybir.dt.bfloat16
    Alu = mybir.AluOpType
    Act = mybir.ActivationFunctionType

    nc = bacc.Bacc("TRN2", target_bir_lowering=False, debug=False,
                   num_devices=N_CORES)

    xs = nc.dram_tensor("xs", [BL, V, T], f16, kind="ExternalInput").ap()
    xs2 = nc.dram_tensor("xs2", [V, BL * T], f16, kind="ExternalInput").ap()
    alt = nc.dram_tensor("alt", [V, BL * V], f16, kind="ExternalInput").ap()
    gp = nc.dram_tensor("gp", [128, NG * 16], f16, kind="ExternalInput").ap()
    atc = nc.dram_tensor("atc", [2, 128, FD2], f16, kind="ExternalInput").ap()
    wq = nc.dram_tensor("wq", [2, 2, 2, 128, 128], f16, kind="ExternalInput").ap()
    smat = nc.dram_tensor("smat", [4, 128, 128], f16, kind="ExternalInput").ap()
    ys = nc.dram_tensor("ys", [2, 128, NG * FD], f16, kind="ExternalOutput").ap()

    with tile.TileContext(nc) as tc:
        import contextlib
        with contextlib.ExitStack() as ctx:
            cpool = ctx.enter_context(tc.tile_pool(name="consts", bufs=1))
            xbpool = ctx.enter_context(tc.tile_pool(name="xbatch", bufs=2))
            wpool = ctx.enter_context(tc.tile_pool(name="work", bufs=4))
            stpool = ctx.enter_context(tc.tile_pool(name="stats", bufs=4))
            pp = ctx.enter_context(tc.tile_pool(name="ps", bufs=1, space="PSUM"))

            # ---- constants (loaded once) ----
            c_at = [cpool.tile([128, FD2], f16, name=f"cat{d}", tag=f"cat{d}")
                    for d in range(2)]
            for d in range(2):
                nc.sync.dma_start(c_at[d][:], atc[d])
            c_wq = [[[cpool.tile([128, 128], f16, name=f"cwq{w}{kh}{F}",
                                 tag=f"cwq{w}{kh}{F}")
                      for F in range(2)] for kh in range(2)] for w in range(2)]
            for w in range(2):
                for kh in range(2):
                    for F in range(2):
                        nc.sync.dma_start(c_wq[w][kh][F][:], wq[w, kh, F])
            c_sm = [cpool.tile([128, 128], f16, name=f"csm{k}", tag=f"csm{k}")
                    for k in range(4)]
            for k in range(4):
                nc.sync.dma_start(c_sm[k][:], smat[k])
            c_gp = cpool.tile([128, NG * 16], f16, name="cgp", tag="cgp")
            nc.sync.dma_start(c_gp[:], gp[:])
            c_alt = cpool.tile([V, BL * V], f16, name="calt", tag="calt")
            nc.sync.dma_start(c_alt[:], alt[:])

            EYE_D, EYE_U, ZS_D, ZS_U = 0, 1, 2, 3

            def pslice(t, c, lo, hi):
                return t[:, 512 * c + lo:512 * c + hi]

            # cross-stage state
            st = {}

            def issue_sxt(p):
                # prefetch transposed x for group pair p (groups 2p, 2p+1);
                # tile layout [128, (q, h, f)] so each group's view is packed
                if p >= NG // 2:
                    return
                # layout [128, (h, q, f)]: one 2-group xbar transpose per h
                # (single queue — concurrent xbar transposes are unsafe)
                st[("sxt", p)] = sxt = wpool.tile(
                    [128, 2 * FD2], f16, name="t02", tag="sxt", bufs=5)
                for h in range(2):
                    nc.sync.dma_start(
                        sxt[:, FD2 * h:FD2 * (h + 1)],
                        xs[2 * p * NB:(2 * p + 2) * NB, :,
                           128 * h:128 * (h + 1)]
                        .rearrange("b v t -> (b v) t"),
                        transpose=True)

            def front(g):
                if g % XG == 0:
                    st[("xng", g // XG)] = xng = xbpool.tile(
                        [V, XG * NB * T], f16, name="t01", tag="xng")
                    nc.scalar.dma_start(
                        xng[:], xs2[:, g * NB * T:(g + XG) * NB * T])
                if g % 2 == 0:
                    issue_sxt(g // 2 + 3)
                # per-group view [128, 2(h), FD] (strided over the q pairing)
                sxt = st[("sxt", g // 2)]
                q = g % 2

                g1r = wpool.tile([128, FD], f16, name="t03", tag="g1r")
                g3r = wpool.tile([128, FD], f16, name="t04", tag="g3r")
                gsl = c_gp[:, g * 16:(g + 1) * 16]
                nc.scalar.copy(
                    g1r[:].rearrange("p (n v) -> p n v", n=NB),
                    gsl[:, 0:NB].unsqueeze(2).broadcast_to([128, NB, V]))
                nc.scalar.copy(
                    g3r[:].rearrange("p (n v) -> p n v", n=NB),
                    gsl[:, NB:16].unsqueeze(2).broadcast_to([128, NB, V]))

                g1s = wpool.tile([128, FD2], f16, name="t05", tag="g1s")
                g3s = wpool.tile([128, FD2], f16, name="t06", tag="g3s")
                ulo = wpool.tile([128, FD2], f16, name="t07", tag="ulo")
                uhi = wpool.tile([128, FD2], f16, name="t08", tag="uhi")
                sxt3 = sxt[:].rearrange("p (h q f) -> p h q f",
                                        h=2, q=2)[:, :, q, :]
                g1b = g1r[:].unsqueeze(1).broadcast_to([128, 2, FD])
                g3b = g3r[:].unsqueeze(1).broadcast_to([128, 2, FD])
                nc.vector.tensor_tensor(
                    g1s[:].rearrange("p (h f) -> p h f", h=2), g1b, sxt3,
                    Alu.mult)
                nc.vector.tensor_tensor(
                    g3s[:].rearrange("p (h f) -> p h f", h=2), g3b, sxt3,
                    Alu.mult)
                nc.vector.tensor_tensor(ulo[:], g3s[:], c_at[0][:], Alu.mult)
                nc.vector.tensor_tensor(uhi[:], g3s[:], c_at[1][:], Alu.mult)
                st[("g1s", g)] = g1s
                st[("ulo", g)] = ulo
                st[("uhi", g)] = uhi

            def stage_a(g):
                ulo, uhi = st.pop(("ulo", g)), st.pop(("uhi", g))
                xng = st[("xng", g // XG)]
                pXM = [pp.tile([128, PW], f32, name="t09", tag="pp", bufs=4)
                       for _ in range(2)]
                xoff = (g % XG) * NB * T
                for h in range(2):
                    for c in range(2):
                        for i in range(4):
                            s = 4 * c + i
                            b = g * NB + s
                            lhs = xng[:, xoff + s * T + 128 * h:
                                      xoff + s * T + 128 * (h + 1)]
                            nc.tensor.matmul(
                                pslice(pXM[h], c, 66 * i, 66 * (i + 1)),
                                lhs, c_alt[:, b * V:(b + 1) * V],
                                start=(i == 0), stop=False,
                                skip_group_check=True)
                        sl_ = slice(FD * h + HC * c, FD * h + HC * (c + 1))
                        out_c = pslice(pXM[h], c, 0, HC)
                        nc.tensor.matmul(out_c, c_sm[EYE_D][:],
                                         ulo[:, sl_], start=False,
                                         stop=False, skip_group_check=True)
                        nc.tensor.matmul(out_c, c_sm[EYE_U][:],
                                         uhi[:, sl_], start=False,
                                         stop=False, skip_group_check=True)
                        if h == 1:
                            osl = slice(HC * c, HC * (c + 1))
                            nc.tensor.matmul(out_c, c_sm[ZS_D][:],
                                             ulo[:, osl], start=False,
                                             stop=True, skip_group_check=True)
                        else:
                            osl = slice(FD + HC * c, FD + HC * (c + 1))
                            nc.tensor.matmul(out_c, c_sm[ZS_U][:],
                                             uhi[:, osl], start=False,
                                             stop=True,
                                             skip_group_check=True)
                xms = wpool.tile([128, FD2], f16, name="t10", tag="xms")
                for h in range(2):
                    nc.scalar.copy(
                        xms[:, FD * h:FD * (h + 1)]
                        .rearrange("p (c f) -> p c f", c=2),
                        pXM[h][:].rearrange("p (c f) -> p c f", c=2)
                        [:, :, 0:HC])
                st[("xms", g)] = xms

            def fc_tail(g):
                xms = st.pop(("xms", g))
                g1s = st.pop(("g1s", g))
                pH = [pp.tile([128, PW], f32, name="t11", tag="pp", bufs=4)
                      for _ in range(2)]
                for F in range(2):
                    for c in range(2):
                        first = True
                        for kh in range(2):
                            sl_ = slice(FD * kh + HC * c,
                                        FD * kh + HC * (c + 1))
                            for w, stream in ((0, xms), (1, g1s)):
                                nc.tensor.matmul(
                                    pslice(pH[F], c, 0, HC),
                                    c_wq[w][kh][F][:],
                                    stream[:, sl_],
                                    start=first,
                                    stop=(kh == 1 and w == 1))
                                first = False

                sH = [wpool.tile([128, FD], f16, name="t12", tag=f"sh{F}")
                      for F in range(2)]
                for F in range(2):
                    pHs = pH[F][:].rearrange("p (c f) -> p c f", c=2)[:, :, 0:HC]
                    dst = sH[F][:].rearrange("p (c f) -> p c f", c=2)
                    if F == 0:
                        nc.scalar.copy(dst, pHs)
                    else:
                        nc.vector.tensor_copy(dst, pHs)
                    nc.gpsimd.dma_start(ys[F, :, g * FD:(g + 1) * FD],
                                        sH[F][:])

            for p0 in range(3):
                issue_sxt(p0)
            for gg in range(NG + 2):
                if gg < NG:
                    front(gg)
                if 1 <= gg <= NG:
                    stage_a(gg - 1)
                if gg >= 2:
                    fc_tail(gg - 2)

    nc.compile()
    _NC_CACHE["nc"] = nc
    return nc


def _gate_np(x, mlp, if_make_dynamic, tau):
    """Replicate the reference gating exactly (jax fp32 on CPU)."""
    import jax
    import jax.numpy as jnp

    xj = jnp.asarray(x)
    prob = xj.mean(axis=1) @ jnp.asarray(mlp)
    if if_make_dynamic:
        u = jax.random.uniform(jax.random.key(42), prob.shape,
                               minval=1e-10, maxval=1.0)
        gumbel = -jnp.log(-jnp.log(u))
        soft = jax.nn.softmax((prob + gumbel) / tau, axis=-1)
        # forward value of (hard + soft - stop_grad(soft)) is exactly hard;
        # return the exact one-hot so 0/1 gates survive integer indexing
        gate = jax.nn.one_hot(jnp.argmax(soft, axis=-1), prob.shape[-1],
                              dtype=soft.dtype)
    else:
        gate = jnp.zeros_like(prob).at[:, 0].set(1.0)
    return np.asarray(gate, dtype=np.float32)


def kernel(x, mlp, adj_j, adj_t, adj_jc, adj_tj, fc_w, fc_b, alpha, beta,
           if_make_dynamic, tau):
    from concourse.bass_utils import run_bass_kernel_spmd

    x = np.asarray(x, dtype=np.float32)
    mlp = np.asarray(mlp, dtype=np.float32)
    adj_j = np.asarray(adj_j, dtype=np.float32)
    adj_t = np.asarray(adj_t, dtype=np.float32)
    adj_jc = np.asarray(adj_jc, dtype=np.float32)
    adj_tj = np.asarray(adj_tj, dtype=np.float32)
    fc_w = np.asarray(fc_w, dtype=np.float32)
    alpha_v = np.asarray(alpha, dtype=np.float32).reshape(V)
    beta_v = np.asarray(beta, dtype=np.float32).reshape(V)

    gate = _gate_np(x, mlp, if_make_dynamic, tau)
    g1, g2, g3 = gate[:, 1], gate[:, 2], gate[:, 3]

    A1 = np.kron(adj_j, np.eye(3, dtype=np.float32))
    A3 = np.zeros((V, V), dtype=np.float32)
    for j in range(J):
        A3[3 * j:3 * j + 3, 3 * j:3 * j + 3] = adj_jc[j]
    alt2 = np.stack([A1.T.copy(), (A1 + A3).T.copy()]).astype(np.float16)
    alt_all = alt2[g2.astype(np.int64)]                   # [B, V, V]

    idx = np.arange(T)
    band = (np.abs(idx[:, None] - idx[None, :]) == 1).astype(np.float32)
    M2 = adj_t * band
    W2 = (fc_w @ M2) * W2SCALE

    wq = np.zeros((2, 2, 2, 128, 128), dtype=np.float32)
    for kh in range(2):
        for F in range(2):
            wq[0, kh, F] = fc_w[128 * F:128 * (F + 1),
                                128 * kh:128 * (kh + 1)].T
            wq[1, kh, F] = W2[128 * F:128 * (F + 1),
                              128 * kh:128 * (kh + 1)].T

    alo_p = np.zeros((T, V), dtype=np.float32)
    ahi_p = np.zeros((T, V), dtype=np.float32)
    alo_p[:T - 1, :] = adj_tj[:, np.arange(1, T), np.arange(0, T - 1)].T
    ahi_p[1:, :] = adj_tj[:, np.arange(0, T - 1), np.arange(1, T)].T
    atc = np.zeros((2, 128, FD2), dtype=np.float32)
    for h in range(2):
        atc[0, :, FD * h:FD * (h + 1)] = np.tile(
            alo_p[128 * h:128 * (h + 1)], (1, NB))
        atc[1, :, FD * h:FD * (h + 1)] = np.tile(
            ahi_p[128 * h:128 * (h + 1)], (1, NB))

    smat = np.zeros((4, 128, 128), dtype=np.float32)
    smat[0] = np.eye(128, k=1)      # EYE_D: out[p] = u[p-1]
    smat[1] = np.eye(128, k=-1)     # EYE_U: out[p] = u[p+1]
    smat[2][127, 0] = 1.0           # ZS_D seam: h1 p0 <- u_lo[h0][127]
    smat[3][0, 127] = 1.0           # ZS_U seam: h0 p127 <- u_hi[h1][0]

    x_f16 = x.astype(np.float16)
    atc_f16 = atc.astype(np.float16)
    wq_f16 = wq.astype(np.float16)
    smat_f16 = smat.astype(np.float16)

    in_maps = []
    for cidx in range(N_CORES):
        sl_ = slice(cidx * BL, (cidx + 1) * BL)
        gp_c = np.zeros((NG, 16), dtype=np.float32)
        gp_c[:, 0:NB] = g1[sl_].reshape(NG, NB) / W2SCALE
        gp_c[:, NB:16] = g3[sl_].reshape(NG, NB)
        gp_full = np.broadcast_to(gp_c.reshape(1, NG * 16),
                                  (128, NG * 16)).astype(np.float16)
        in_maps.append(dict(
            xs=np.ascontiguousarray(x_f16[sl_]),
            xs2=np.ascontiguousarray(
                x_f16[sl_].transpose(1, 0, 2)).reshape(V, BL * T),
            alt=np.ascontiguousarray(
                alt_all[sl_].transpose(1, 0, 2)).reshape(V, BL * V),
            gp=gp_full, atc=atc_f16, wq=wq_f16, smat=smat_f16,
        ))

    nc = _build_nc()
    res = run_bass_kernel_spmd(nc, in_maps, core_ids=list(range(N_CORES)),
                               **_RUN_KW)
    _LAST_RES.clear()
    _LAST_RES["res"] = res

    # host tail: LN stats from shipped fp16 h (centered, fp32, clamped),
    # then y = x + (h - m)/sqrt(var+eps) * alpha + beta
    out = np.empty((B, V, T), dtype=np.float32)
    for cidx in range(N_CORES):
        yt = np.asarray(res.results[cidx]["ys"]).astype(np.float32)
        h = yt.reshape(2, 128, NG, NB, V).transpose(2, 3, 4, 0, 1) \
            .reshape(BL, V, T)
        m = h.mean(axis=1, keepdims=True)
        var = h.var(axis=1, keepdims=True)
        nv = (h - m) / np.sqrt(var + 1e-5)
        nv = nv * alpha_v[None, :, None] + beta_v[None, :, None]
        out[cidx * BL:(cidx + 1) * BL] = x[cidx * BL:(cidx + 1) * BL] + nv
    return out


_RUN_KW = {}
_LAST_RES = {}


# revision 24
# speedup vs baseline: 1.7246x; 1.2354x over previous
"""Trainium2 Bass kernel for nn_GCBlock (gnn_message_passing) — v5.

fp16 matmul pipeline in a transposed (time-on-partition) layout:
- joint-mix (g2 folded into per-sample AL^T, 2 host variants), adj_t band
  folded into W2 = fc_w @ M2 as a second FC stream on g1*xT, adj_tj band
  via shift-commuted coefficient prefold with shift matmuls accumulating
  straight into the joint-mix PSUM.
- 3-stage software-pipelined emission: front(g) / stageA(g-1) / fc+tail(g-2)
  per emission step, so every engine queue always holds ready work and the
  PE stream stays dense (HAM warm).
- xT via xbar DMA-transpose, batched 2 groups per instruction on a single
  HWDGE queue (concurrent xbar transposes are unsafe), prefetched one pair
  ahead.
- device ships h (fp16, transposed); the host computes LN stats (centered,
  fp32) and applies y = x + (h - m)/sqrt(var+eps)*alpha + beta. This keeps
  the LN tail off the device and the residual in full fp32.
- gate is the exact one-hot forward value (hard + soft - stop_grad(soft)
  == hard), so 0/1 gates survive integer indexing on the host.
"""
import numpy as np

B, V, T, J = 2048, 66, 256, 22
N_CORES = 8
BL = B // N_CORES          # 256 samples per core
NB = 8                     # samples per group
NG = BL // NB              # 32 groups
FD = NB * V                # 528 batched free dim
FD2 = 2 * FD               # both time halves side by side
HC = FD // 2               # 264 per col-half
PW = 1024                  # psum tile width (2 banks); c-blocks at 0 / 512
XG = 4                     # groups per natural-x batch DMA
W2SCALE = 64.0             # host-folds: W2 *= 64, g1 pattern /= 64

_NC_CACHE = {}


def _build_nc():
    if "nc" in _NC_CACHE:
        return _NC_CACHE["nc"]
    import concourse.bacc as bacc
    import concourse.mybir as mybir
    import concourse.tile as tile

    f32 = mybir.dt.float32
    f16 = mybir.dt.float16
    bf16 = mybir.dt.bfloat16
    Alu = mybir.AluOpType
    Act = mybir.ActivationFunctionType

    nc = bacc.Bacc("TRN2", target_bir_lowering=False, debug=False,
                   num_devices=N_CORES)

    xt = nc.dram_tensor("xt", [2, 128, NG * FD], f16, kind="ExternalInput").ap()
    xs2 = nc.dram_tensor("xs2", [V, BL * T], f16, kind="ExternalInput").ap()
    alt = nc.dram_tensor("alt", [V, BL * V], f16, kind="ExternalInput").ap()
    gp = nc.dram_tensor("gp", [128, NG * 16], f16, kind="ExternalInput").ap()
    atc = nc.dram_tensor("atc", [2, 128, FD2], f16, kind="ExternalInput").ap()
    wq = nc.dram_tensor("wq", [2, 2, 2, 128, 128], f16, kind="ExternalInput").ap()
    smat = nc.dram_tensor("smat", [4, 128, 128], f16, kind="ExternalInput").ap()
    ys = nc.dram_tensor("ys", [2, 128, NG * FD], f16, kind="ExternalOutput").ap()

    with tile.TileContext(nc) as tc:
        import contextlib
        with contextlib.ExitStack() as ctx:
            cpool = ctx.enter_context(tc.tile_pool(name="consts", bufs=1))
            xbpool = ctx.enter_context(tc.tile_pool(name="xbatch", bufs=2))
            wpool = ctx.enter_context(tc.tile_pool(name="work", bufs=4))
            stpool = ctx.enter_context(tc.tile_pool(name="stats", bufs=4))
            pp = ctx.enter_context(tc.tile_pool(name="ps", bufs=1, space="PSUM"))

            # ---- constants (loaded once) ----
            c_at = [cpool.tile([128, FD2], f16, name=f"cat{d}", tag=f"cat{d}")
                    for d in range(2)]
            for d in range(2):
                nc.sync.dma_start(c_at[d][:], atc[d])
            c_wq = [[[cpool.tile([128, 128], f16, name=f"cwq{w}{kh}{F}",
                                 tag=f"cwq{w}{kh}{F}")
                      for F in range(2)] for kh in range(2)] for w in range(2)]
            for w in range(2):
                for kh in range(2):
                    for F in range(2):
                        nc.sync.dma_start(c_wq[w][kh][F][:], wq[w, kh, F])
            c_sm = [cpool.tile([128, 128], f16, name=f"csm{k}", tag=f"csm{k}")
                    for k in range(4)]
            for k in range(4):
                nc.sync.dma_start(c_sm[k][:], smat[k])
            c_gp = cpool.tile([128, NG * 16], f16, name="cgp", tag="cgp")
            nc.sync.dma_start(c_gp[:], gp[:])
            c_alt = cpool.tile([V, BL * V], f16, name="calt", tag="calt")
            nc.sync.dma_start(c_alt[:], alt[:])

            EYE_D, EYE_U, ZS_D, ZS_U = 0, 1, 2, 3

            def pslice(t, c, lo, hi):
                return t[:, 512 * c + lo:512 * c + hi]

            # cross-stage state
            st = {}

            def issue_sxt(p):
                # prefetch transposed x for group pair p (groups 2p, 2p+1);
                # tile layout [128, (q, h, f)] so each group's view is packed
                if p >= NG // 2:
                    return
                # layout [128, (h, q, f)]: host-pretransposed xT, plain
                # contiguous DMA per h (no xbar, full HBM rate)
                st[("sxt", p)] = sxt = wpool.tile(
                    [128, 2 * FD2], f16, name="t02", tag="sxt", bufs=5)
                for h in range(2):
                    nc.sync.dma_start(
                        sxt[:, FD2 * h:FD2 * (h + 1)],
                        xt[h, :, 2 * p * FD:(2 * p + 2) * FD])

            def front(g):
                if g % XG == 0:
                    st[("xng", g // XG)] = xng = xbpool.tile(
                        [V, XG * NB * T], f16, name="t01", tag="xng")
                    nc.scalar.dma_start(
                        xng[:], xs2[:, g * NB * T:(g + XG) * NB * T])
                if g % 2 == 0:
                    issue_sxt(g // 2 + 3)
                # per-group view [128, 2(h), FD] (strided over the q pairing)
                sxt = st[("sxt", g // 2)]
                q = g % 2

                g1r = wpool.tile([128, FD], f16, name="t03", tag="g1r")
                g3r = wpool.tile([128, FD], f16, name="t04", tag="g3r")
                gsl = c_gp[:, g * 16:(g + 1) * 16]
                nc.scalar.copy(
                    g1r[:].rearrange("p (n v) -> p n v", n=NB),
                    gsl[:, 0:NB].unsqueeze(2).broadcast_to([128, NB, V]))
                nc.scalar.copy(
                    g3r[:].rearrange("p (n v) -> p n v", n=NB),
                    gsl[:, NB:16].unsqueeze(2).broadcast_to([128, NB, V]))

                g1s = wpool.tile([128, FD2], f16, name="t05", tag="g1s")
                g3s = wpool.tile([128, FD2], f16, name="t06", tag="g3s")
                ulo = wpool.tile([128, FD2], f16, name="t07", tag="ulo")
                uhi = wpool.tile([128, FD2], f16, name="t08", tag="uhi")
                sxt3 = sxt[:].rearrange("p (h q f) -> p h q f",
                                        h=2, q=2)[:, :, q, :]
                g1b = g1r[:].unsqueeze(1).broadcast_to([128, 2, FD])
                g3b = g3r[:].unsqueeze(1).broadcast_to([128, 2, FD])
                nc.vector.tensor_tensor(
                    g1s[:].rearrange("p (h f) -> p h f", h=2), g1b, sxt3,
                    Alu.mult)
                nc.vector.tensor_tensor(
                    g3s[:].rearrange("p (h f) -> p h f", h=2), g3b, sxt3,
                    Alu.mult)
                nc.vector.tensor_tensor(ulo[:], g3s[:], c_at[0][:], Alu.mult)
                nc.vector.tensor_tensor(uhi[:], g3s[:], c_at[1][:], Alu.mult)
                st[("g1s", g)] = g1s
                st[("ulo", g)] = ulo
                st[("uhi", g)] = uhi

            def stage_a(g):
                ulo, uhi = st.pop(("ulo", g)), st.pop(("uhi", g))
                xng = st[("xng", g // XG)]
                pXM = [pp.tile([128, PW], f32, name="t09", tag="pp", bufs=4)
                       for _ in range(2)]
                xoff = (g % XG) * NB * T
                for h in range(2):
                    for c in range(2):
                        for i in range(4):
                            s = 4 * c + i
                            b = g * NB + s
                            lhs = xng[:, xoff + s * T + 128 * h:
                                      xoff + s * T + 128 * (h + 1)]
                            nc.tensor.matmul(
                                pslice(pXM[h], c, 66 * i, 66 * (i + 1)),
                                lhs, c_alt[:, b * V:(b + 1) * V],
                                start=(i == 0), stop=False,
                                skip_group_check=True)
                        sl_ = slice(FD * h + HC * c, FD * h + HC * (c + 1))
                        out_c = pslice(pXM[h], c, 0, HC)
                        nc.tensor.matmul(out_c, c_sm[EYE_D][:],
                                         ulo[:, sl_], start=False,
                                         stop=False, skip_group_check=True)
                        nc.tensor.matmul(out_c, c_sm[EYE_U][:],
                                         uhi[:, sl_], start=False,
                                         stop=False, skip_group_check=True)
                        if h == 1:
                            osl = slice(HC * c, HC * (c + 1))
                            nc.tensor.matmul(out_c, c_sm[ZS_D][:],
                                             ulo[:, osl], start=False,
                                             stop=True, skip_group_check=True)
                        else:
                            osl = slice(FD + HC * c, FD + HC * (c + 1))
                            nc.tensor.matmul(out_c, c_sm[ZS_U][:],
                                             uhi[:, osl], start=False,
                                             stop=True,
                                             skip_group_check=True)
                xms = wpool.tile([128, FD2], f16, name="t10", tag="xms")
                for h in range(2):
                    nc.scalar.copy(
                        xms[:, FD * h:FD * (h + 1)]
                        .rearrange("p (c f) -> p c f", c=2),
                        pXM[h][:].rearrange("p (c f) -> p c f", c=2)
                        [:, :, 0:HC])
                st[("xms", g)] = xms

            def fc_tail(g):
                xms = st.pop(("xms", g))
                g1s = st.pop(("g1s", g))
                pH = [pp.tile([128, PW], f32, name="t11", tag="pp", bufs=4)
                      for _ in range(2)]
                for F in range(2):
                    for c in range(2):
                        first = True
                        for kh in range(2):
                            sl_ = slice(FD * kh + HC * c,
                                        FD * kh + HC * (c + 1))
                            for w, stream in ((0, xms), (1, g1s)):
                                nc.tensor.matmul(
                                    pslice(pH[F], c, 0, HC),
                                    c_wq[w][kh][F][:],
                                    stream[:, sl_],
                                    start=first,
                                    stop=(kh == 1 and w == 1))
                                first = False

                sH = [wpool.tile([128, FD], f16, name="t12", tag=f"sh{F}")
                      for F in range(2)]
                for F in range(2):
                    pHs = pH[F][:].rearrange("p (c f) -> p c f", c=2)[:, :, 0:HC]
                    dst = sH[F][:].rearrange("p (c f) -> p c f", c=2)
                    if F == 0:
                        nc.scalar.copy(dst, pHs)
                    else:
                        nc.vector.tensor_copy(dst, pHs)
                    nc.gpsimd.dma_start(ys[F, :, g * FD:(g + 1) * FD],
                                        sH[F][:])

            for p0 in range(3):
                issue_sxt(p0)
            for gg in range(NG + 2):
                if gg < NG:
                    front(gg)
                if 1 <= gg <= NG:
                    stage_a(gg - 1)
                if gg >= 2:
                    fc_tail(gg - 2)

    nc.compile()
    _NC_CACHE["nc"] = nc
    return nc


def _gate_np(x, mlp, if_make_dynamic, tau):
    """Replicate the reference gating exactly (jax fp32 on CPU)."""
    import jax
    import jax.numpy as jnp

    xj = jnp.asarray(x)
    prob = xj.mean(axis=1) @ jnp.asarray(mlp)
    if if_make_dynamic:
        u = jax.random.uniform(jax.random.key(42), prob.shape,
                               minval=1e-10, maxval=1.0)
        gumbel = -jnp.log(-jnp.log(u))
        soft = jax.nn.softmax((prob + gumbel) / tau, axis=-1)
        # forward value of (hard + soft - stop_grad(soft)) is exactly hard;
        # return the exact one-hot so 0/1 gates survive integer indexing
        gate = jax.nn.one_hot(jnp.argmax(soft, axis=-1), prob.shape[-1],
                              dtype=soft.dtype)
    else:
        gate = jnp.zeros_like(prob).at[:, 0].set(1.0)
    return np.asarray(gate, dtype=np.float32)


def kernel(x, mlp, adj_j, adj_t, adj_jc, adj_tj, fc_w, fc_b, alpha, beta,
           if_make_dynamic, tau):
    from concourse.bass_utils import run_bass_kernel_spmd

    x = np.asarray(x, dtype=np.float32)
    mlp = np.asarray(mlp, dtype=np.float32)
    adj_j = np.asarray(adj_j, dtype=np.float32)
    adj_t = np.asarray(adj_t, dtype=np.float32)
    adj_jc = np.asarray(adj_jc, dtype=np.float32)
    adj_tj = np.asarray(adj_tj, dtype=np.float32)
    fc_w = np.asarray(fc_w, dtype=np.float32)
    alpha_v = np.asarray(alpha, dtype=np.float32).reshape(V)
    beta_v = np.asarray(beta, dtype=np.float32).reshape(V)

    gate = _gate_np(x, mlp, if_make_dynamic, tau)
    g1, g2, g3 = gate[:, 1], gate[:, 2], gate[:, 3]

    A1 = np.kron(adj_j, np.eye(3, dtype=np.float32))
    A3 = np.zeros((V, V), dtype=np.float32)
    for j in range(J):
        A3[3 * j:3 * j + 3, 3 * j:3 * j + 3] = adj_jc[j]
    alt2 = np.stack([A1.T.copy(), (A1 + A3).T.copy()]).astype(np.float16)
    alt_all = alt2[g2.astype(np.int64)]                   # [B, V, V]

    idx = np.arange(T)
    band = (np.abs(idx[:, None] - idx[None, :]) == 1).astype(np.float32)
    M2 = adj_t * band
    W2 = (fc_w @ M2) * W2SCALE

    wq = np.zeros((2, 2, 2, 128, 128), dtype=np.float32)
    for kh in range(2):
        for F in range(2):
            wq[0, kh, F] = fc_w[128 * F:128 * (F + 1),
                                128 * kh:128 * (kh + 1)].T
            wq[1, kh, F] = W2[128 * F:128 * (F + 1),
                              128 * kh:128 * (kh + 1)].T

    alo_p = np.zeros((T, V), dtype=np.float32)
    ahi_p = np.zeros((T, V), dtype=np.float32)
    alo_p[:T - 1, :] = adj_tj[:, np.arange(1, T), np.arange(0, T - 1)].T
    ahi_p[1:, :] = adj_tj[:, np.arange(0, T - 1), np.arange(1, T)].T
    atc = np.zeros((2, 128, FD2), dtype=np.float32)
    for h in range(2):
        atc[0, :, FD * h:FD * (h + 1)] = np.tile(
            alo_p[128 * h:128 * (h + 1)], (1, NB))
        atc[1, :, FD * h:FD * (h + 1)] = np.tile(
            ahi_p[128 * h:128 * (h + 1)], (1, NB))

    smat = np.zeros((4, 128, 128), dtype=np.float32)
    smat[0] = np.eye(128, k=1)      # EYE_D: out[p] = u[p-1]
    smat[1] = np.eye(128, k=-1)     # EYE_U: out[p] = u[p+1]
    smat[2][127, 0] = 1.0           # ZS_D seam: h1 p0 <- u_lo[h0][127]
    smat[3][0, 127] = 1.0           # ZS_U seam: h0 p127 <- u_hi[h1][0]

    x_f16 = x.astype(np.float16)
    atc_f16 = atc.astype(np.float16)
    wq_f16 = wq.astype(np.float16)
    smat_f16 = smat.astype(np.float16)

    in_maps = []
    for cidx in range(N_CORES):
        sl_ = slice(cidx * BL, (cidx + 1) * BL)
        gp_c = np.zeros((NG, 16), dtype=np.float32)
        gp_c[:, 0:NB] = g1[sl_].reshape(NG, NB) / W2SCALE
        gp_c[:, NB:16] = g3[sl_].reshape(NG, NB)
        gp_full = np.broadcast_to(gp_c.reshape(1, NG * 16),
                                  (128, NG * 16)).astype(np.float16)
        in_maps.append(dict(
            xt=np.ascontiguousarray(
                x_f16[sl_].transpose(2, 0, 1)).reshape(2, 128, NG * FD),
            xs2=np.ascontiguousarray(
                x_f16[sl_].transpose(1, 0, 2)).reshape(V, BL * T),
            alt=np.ascontiguousarray(
                alt_all[sl_].transpose(1, 0, 2)).reshape(V, BL * V),
            gp=gp_full, atc=atc_f16, wq=wq_f16, smat=smat_f16,
        ))

    nc = _build_nc()
    res = run_bass_kernel_spmd(nc, in_maps, core_ids=list(range(N_CORES)),
                               **_RUN_KW)
    _LAST_RES.clear()
    _LAST_RES["res"] = res

    # host tail: LN stats from shipped fp16 h (centered, fp32, clamped),
    # then y = x + (h - m)/sqrt(var+eps) * alpha + beta
    out = np.empty((B, V, T), dtype=np.float32)
    for cidx in range(N_CORES):
        yt = np.asarray(res.results[cidx]["ys"]).astype(np.float32)
        h = yt.reshape(2, 128, NG, NB, V).transpose(2, 3, 4, 0, 1) \
            .reshape(BL, V, T)
        m = h.mean(axis=1, keepdims=True)
        var = h.var(axis=1, keepdims=True)
        nv = (h - m) / np.sqrt(var + 1e-5)
        nv = nv * alpha_v[None, :, None] + beta_v[None, :, None]
        out[cidx * BL:(cidx + 1) * BL] = x[cidx * BL:(cidx + 1) * BL] + nv
    return out


_RUN_KW = {}
_LAST_RES = {}
